# revision 18
# baseline (speedup 1.0000x reference)
"""Trainium2 Bass kernel for nn_Lowpass: per-128-block RBJ lowpass biquad.

Algorithm (per 128-sample block, zero initial state):
  y = IIR(FIR(x)) with per-block coefficients from avg-pooled control params.
  FIR: u[n] = x[n] + 2 x[n-1] + x[n-2]    (b0 factored out; b2 == b0, b1 == 2 b0)
  IIR poles are complex (r e^{+-i theta}).  Rotated-frame decomposition turns
  the order-2 recurrence into two real first-order scans that map directly to
  the DVE tensor_tensor_scan instruction:
      v_re[n] = r v_re[n-1] + cos(n theta) u[n]
      v_im[n] = r v_im[n-1] - sin(n theta) u[n]
      y[n]    = Z b0 (cos(n theta + phi) v_re[n] - sin(n theta + phi) v_im[n])
  with 2c = 1 - i pr/pi the pole residue, Z = |2c|, phi = arg(2c).
  (The kernel scans d_im = +sin * u, flipping the recombine sign to +.)

Work distribution:
  PE:  angle grids n*theta/2pi (outer product theta'^T @ blockdiag(iota)),
       coefficient transposes.
  ACT: control-param avg-pooling (Abs + accum_out), magic-number rounding
       bias, all sin/cos/arctan evaluations (single trig table set).
  DVE: FIR, range-reduce subtract, the two scans, recombine multiplies,
       small coefficient arithmetic (sqrt via seeded Newton, no table switch).

Sharding: pure data parallel, core c processes batches [4c, 4c+4).
"""

import sys

sys.path.insert(0, "/opt/trn_rl_repo")

import math
from contextlib import ExitStack

import numpy as np

import concourse.bacc as bacc
import concourse.bass as bass
import concourse.mybir as mybir
from concourse.tile import TileContext

F32 = mybir.dt.float32
AX = mybir.AxisListType
ALU = mybir.AluOpType
ACT = mybir.ActivationFunctionType

SR = 44100.0
BLOCK = 128
FC_MIN, FC_MAX = 2000.0, 20000.0
Q_MIN, Q_MAX = 0.1, 10.0
PI = math.pi
MAGIC = 1.5 * 2.0 ** 23     # fp32 round-to-nearest-int bias
INV2PI = 1.0 / (2.0 * PI)
TWOPI = 2.0 * PI
# linear minimax seed for sqrt on [0.70, 0.87] (one Newton step after)
SQ_C1 = 0.5672
SQ_C0 = 0.4402


def make_consts(NB, S):
    """Host-precomputed constants: block-diag iota/ones rhs + identity."""
    P = 128
    F = S // P
    HI = F // BLOCK
    rhs = np.zeros((64, F), np.float32)
    for g in range(2):
        for h in range(HI):
            rhs[32 * g + h, h * BLOCK:(h + 1) * BLOCK] = np.arange(
                BLOCK, dtype=np.float32)
            rhs[32 * g + 16 + h, h * BLOCK:(h + 1) * BLOCK] = 1.0
    ident = np.eye(128, dtype=np.float32)
    return {"rhs_c": rhs, "ident": ident}


def build_core_kernel(NB=4, S=262144, n_devices=8, dev_clamp=False):
    """Bass kernel for one core: NB batches of S samples."""
    P = 128
    F = S // P            # free elems per row (per batch)
    HI = F // BLOCK       # blocks per partition row
    nblk = S // BLOCK     # blocks per batch
    NBG = (NB + 1) // 2   # coeff groups of 2 batches

    nc = bacc.Bacc("TRN2", target_bir_lowering=False, debug=False,
                   num_devices=n_devices)
    x_d = nc.dram_tensor("x", [NB, S], F32, kind="ExternalInput")
    cp_d = nc.dram_tensor("cp", [NB, 2, S], F32, kind="ExternalInput")
    rhs_d = nc.dram_tensor("rhs_c", [64, F], F32, kind="ExternalInput")
    id_d = nc.dram_tensor("ident", [128, 128], F32, kind="ExternalInput")
    y_d = nc.dram_tensor("y", [NB, S], F32, kind="ExternalOutput")
    fc_d = nc.dram_tensor("fc", [NB, nblk], F32, kind="ExternalOutput")
    q_d = nc.dram_tensor("q", [NB, nblk], F32, kind="ExternalOutput")

    with TileContext(nc) as tc, ExitStack() as ctx:
        cpool = ctx.enter_context(tc.tile_pool(name="const", bufs=1))
        spool = ctx.enter_context(tc.tile_pool(name="small", bufs=2))
        big = ctx.enter_context(tc.tile_pool(name="big", bufs=2))
        psum = ctx.enter_context(tc.tile_pool(name="psum", bufs=1,
                                              space="PSUM"))

        rhs_sb = cpool.tile([64, F], F32, tag="rhs_sb")
        nc.sync.dma_start(out=rhs_sb[:], in_=rhs_d[:, :])
        id_sb = cpool.tile([128, 128], F32, tag="id_sb")
        nc.sync.dma_start(out=id_sb[:], in_=id_d[:, :])

        _consts = {}

        def c_ap(val):
            if val not in _consts:
                t = cpool.tile([P, 1], F32, tag=f"c{len(_consts)}",
                               name=f"c{len(_consts)}")
                nc.vector.memset(t[:], val)
                _consts[val] = t
            return _consts[val][:]

        _bufs2 = {"x", "ang", "cpg", "spg", "cpt", "trash"}

        def bt(tag, shape=None):
            return big.tile(shape or [P, F], F32, tag=tag, name=tag,
                            bufs=2 if tag in _bufs2 else 1)

        # ---------- per-group coefficient pipeline ----------
        # group g covers batches 2g, 2g+1; W lanes per group per partition
        lhsT_g = []     # per group: (64,128) rows 32*(b%2)+[0:16]=theta',
        #                 +[16:32]=phi' (in turns)
        r_all, zb_all = [], []
        for g in range(NBG):
            bs = list(range(2 * g, min(2 * g + 2, NB)))
            W = HI * len(bs)

            def gt(tag):
                return spool.tile([P, W], F32, tag=tag, name=tag, bufs=2)

            s0, s1 = gt("s0"), gt("s1")
            for bi, b in enumerate(bs):
                for prm in range(2):
                    cpt = bt("cpt")
                    nc.sync.dma_start(
                        out=cpt[:],
                        in_=cp_d[b, prm].rearrange("(p f) -> p f", p=P))
                    dst = (s0 if prm == 0 else s1)
                    trash = bt("trash", [P, BLOCK])
                    for h in range(HI):
                        nc.scalar.activation(
                            trash[:], cpt[:, h * BLOCK:(h + 1) * BLOCK],
                            ACT.Abs,
                            accum_out=dst[:, bi * HI + h:bi * HI + h + 1])

            fc = gt("fc")
            nc.vector.tensor_scalar(fc[:], s0[:], (FC_MAX - FC_MIN) / BLOCK,
                                    FC_MIN, op0=ALU.mult, op1=ALU.add)
            q = gt("q")
            nc.vector.tensor_scalar(q[:], s1[:], (Q_MAX - Q_MIN) / BLOCK,
                                    Q_MIN, op0=ALU.mult, op1=ALU.add)
            for bi, b in enumerate(bs):
                nc.sync.dma_start(
                    out=fc_d[b].rearrange("(p h) -> p h", p=P),
                    in_=fc[:, bi * HI:(bi + 1) * HI])
                nc.sync.dma_start(
                    out=q_d[b].rearrange("(p h) -> p h", p=P),
                    in_=q[:, bi * HI:(bi + 1) * HI])

            w0 = gt("w0")
            nc.vector.tensor_scalar(
                w0[:], s0[:], (FC_MAX - FC_MIN) / BLOCK * 2.0 * PI / SR,
                FC_MIN * 2.0 * PI / SR, op0=ALU.mult, op1=ALU.add)
            sinw = gt("sinw")
            nc.scalar.activation(sinw[:], w0[:], ACT.Sin)
            cosw = gt("cosw")
            nc.scalar.activation(cosw[:], w0[:], ACT.Sin, scale=-1.0,
                                 bias=c_ap(PI / 2))

            qr = gt("qr")
            nc.vector.reciprocal(qr[:], q[:])
            alpha = gt("alpha")
            nc.vector.scalar_tensor_tensor(alpha[:], sinw[:], 0.5, qr[:],
                                           op0=ALU.mult, op1=ALU.mult)
            t0 = gt("t0")
            nc.vector.tensor_scalar_add(t0[:], alpha[:], 1.0)
            a0r = gt("a0r")
            nc.vector.reciprocal(a0r[:], t0[:])
            t1 = gt("t1")
            nc.vector.tensor_scalar(t1[:], cosw[:], -0.5, 0.5,
                                    op0=ALU.mult, op1=ALU.add)
            b0c = gt("b0c")
            nc.vector.tensor_tensor(b0c[:], t1[:], a0r[:], op=ALU.mult)
            prc = gt("prc")
            nc.vector.tensor_tensor(prc[:], cosw[:], a0r[:], op=ALU.mult)
            t2 = gt("t2")
            nc.vector.tensor_scalar(t2[:], alpha[:], -1.0, 1.0,
                                    op0=ALU.mult, op1=ALU.add)
            a2 = gt("a2")
            nc.vector.tensor_tensor(a2[:], t2[:], a0r[:], op=ALU.mult)
            prsq = gt("prsq")
            nc.vector.tensor_tensor(prsq[:], prc[:], prc[:], op=ALU.mult)
            pi2 = gt("pi2")
            nc.vector.tensor_tensor(pi2[:], a2[:], prsq[:], op=ALU.subtract)

            def sqrt_nr(dst_tag, a):
                # seeded Newton sqrt, valid on [0.70, 0.87]
                y0 = gt(dst_tag + "0")
                nc.vector.tensor_scalar(y0[:], a[:], SQ_C1, SQ_C0,
                                        op0=ALU.mult, op1=ALU.add)
                rc = gt(dst_tag + "r")
                nc.vector.reciprocal(rc[:], y0[:])
                th = gt(dst_tag + "h")
                nc.vector.scalar_tensor_tensor(th[:], a[:], 0.5, rc[:],
                                               op0=ALU.mult, op1=ALU.mult)
                out = gt(dst_tag)
                nc.vector.scalar_tensor_tensor(out[:], y0[:], 0.5, th[:],
                                               op0=ALU.mult, op1=ALU.add)
                return out

            r_t = sqrt_nr("rt", a2)
            pi_ = sqrt_nr("pit", pi2)
            pir = gt("pir")
            nc.vector.reciprocal(pir[:], pi_[:])
            ratio = gt("ratio")
            nc.vector.tensor_tensor(ratio[:], prc[:], pir[:], op=ALU.mult)
            atn = gt("atn")
            nc.scalar.activation(atn[:], ratio[:], ACT.Arctan, scale=-1.0)
            theta = gt("theta")
            nc.vector.tensor_scalar_add(theta[:], atn[:], PI / 2)
            cphi = gt("cphi")
            nc.scalar.activation(cphi[:], atn[:], ACT.Sin, bias=c_ap(PI / 2))
            z_t = gt("z_t")
            nc.vector.reciprocal(z_t[:], cphi[:])
            zb = gt("zb")
            nc.vector.tensor_tensor(zb[:], z_t[:], b0c[:], op=ALU.mult)
            r_all.append(r_t)
            zb_all.append(zb)

            # pack theta'/phi' (turns) for PE transpose; batch bi at cols
            # 32*bi + [0:HI) and 32*bi + 16 + [0:HI)
            tpin = spool.tile([128, 128], F32, tag="tpin", name="tpin",
                              bufs=2)
            nc.vector.memset(tpin[:], 0.0)
            for bi in range(len(bs)):
                nc.vector.tensor_scalar_mul(
                    tpin[:, 32 * bi:32 * bi + HI],
                    theta[:, bi * HI:(bi + 1) * HI], INV2PI)
                nc.vector.tensor_scalar_mul(
                    tpin[:, 32 * bi + 16:32 * bi + 16 + HI],
                    atn[:, bi * HI:(bi + 1) * HI], INV2PI)
            ps_t = psum.tile([128, 128], F32, tag="pa", name="ps_t")
            nc.tensor.transpose(ps_t[:], tpin[:], id_sb[:])
            lt = cpool.tile([64, 128], F32, tag=f"lhsT{g}", name=f"lhsT{g}")
            nc.scalar.copy(lt[:], ps_t[0:64, :])
            lhsT_g.append(lt)

        # ---------- per-batch streaming filter ----------
        for b in range(NB):
            g, bi = b // 2, b % 2
            gsl = slice(bi * HI, (bi + 1) * HI)
            lt = lhsT_g[g]
            p0 = 32 * bi

            x_sb = bt("x")
            nc.sync.dma_start(
                out=x_sb[:], in_=x_d[b].rearrange("(p f) -> p f", p=P))
            xv = x_sb[:].rearrange("p (h t) -> p h t", t=BLOCK)

            # FIR u = x + 2 x_{-1} + x_{-2} (per block; fix cols 0,1)
            u1 = bt("u1")
            nc.vector.scalar_tensor_tensor(
                u1[:, 1:], x_sb[:, :F - 1], 2.0, x_sb[:, 1:],
                op0=ALU.mult, op1=ALU.add)
            nc.vector.tensor_copy(u1[:, 0:1], x_sb[:, 0:1])
            u = bt("u")
            nc.vector.tensor_tensor(u[:, 2:], u1[:, 2:], x_sb[:, :F - 2],
                                    op=ALU.add)
            uv = u[:].rearrange("p (h t) -> p h t", t=BLOCK)
            nc.vector.tensor_copy(uv[:, :, 0:1], xv[:, :, 0:1])
            nc.vector.scalar_tensor_tensor(
                uv[:, :, 1:2], xv[:, :, 0:1], 2.0, xv[:, :, 1:2],
                op0=ALU.mult, op1=ALU.add)

            pa = psum.tile([P, F], F32, tag="pa", name="pa")
            pp = psum.tile([P, F], F32, tag="pp", name="pp")
            for c in range(0, F, 512):
                ce = min(c + 512, F)
                nc.tensor.matmul(pa[:, c:ce], lt[p0:p0 + 16, :],
                                 rhs_sb[p0:p0 + 16, c:ce])
                nc.tensor.matmul(pp[:, c:ce], lt[p0:p0 + 32, :],
                                 rhs_sb[p0:p0 + 32, c:ce])

            def grids(src, ctag, stag):
                # tk = MAGIC + round(src);  gn = (tk - MAGIC) - src = -frac
                tk = bt("tk")
                nc.scalar.activation(tk[:], src[:], ACT.Abs,
                                     bias=c_ap(MAGIC))
                gn = bt("g_" + stag)
                nc.vector.scalar_tensor_tensor(gn[:], tk[:], -MAGIC, src[:],
                                               op0=ALU.add, op1=ALU.subtract)
                if dev_clamp:
                    nc.vector.tensor_scalar(gn[:], gn[:], -0.5, 0.5,
                                            op0=ALU.max, op1=ALU.min)
                # sin(2 pi frac) = sin(-2 pi gn); cos = sin(pi/2 - 2 pi |gn|)
                sgr = bt(stag)
                nc.scalar.activation(sgr[:], gn[:], ACT.Sin, scale=-TWOPI)
                fa = bt("fa")
                nc.scalar.activation(fa[:], gn[:], ACT.Abs)
                cgr = bt(ctag)
                nc.scalar.activation(cgr[:], fa[:], ACT.Sin, scale=-TWOPI,
                                     bias=c_ap(PI / 2))
                return cgr, sgr

            cg, sg = grids(pa, "cg", "sg")      # cos/sin(n theta)
            cpg, spg = grids(pp, "cpg", "spg")  # cos/sin(n theta + phi)

            # scan multiplier grid: r per lane, 0 at block starts
            d0 = bt("d0")
            d0v = d0[:].rearrange("p (h t) -> p h t", t=BLOCK)
            r_b = r_all[g][:, gsl].unsqueeze(2).broadcast_to((P, HI, BLOCK))
            nc.scalar.activation(d0v, r_b, ACT.Copy)
            nc.vector.memset(d0v[:, :, 0:1], 0.0)

            dre = bt("dre")
            nc.vector.tensor_tensor(dre[:], cg[:], u[:], op=ALU.mult)
            dim = bt("dim")
            nc.vector.tensor_tensor(dim[:], sg[:], u[:], op=ALU.mult)

            vre = bt("vre")
            nc.vector.tensor_tensor_scan(vre[:], d0[:], dre[:], 0.0,
                                         op0=ALU.mult, op1=ALU.add)
            vim = bt("vim")
            nc.vector.tensor_tensor_scan(vim[:], d0[:], dim[:], 0.0,
                                         op0=ALU.mult, op1=ALU.add)

            # y = ZB * (cos(psi) v_re + sin(psi) v_im')   [v_im' = -v_im]
            m1 = bt("dre")
            nc.vector.tensor_tensor(m1[:], cpg[:], vre[:], op=ALU.mult)
            m2 = bt("dim")
            nc.vector.tensor_tensor(m2[:], spg[:], vim[:], op=ALU.mult)
            s = bt("u")
            nc.vector.tensor_tensor(s[:], m1[:], m2[:], op=ALU.add)
            y = bt("ang")
            zb_b = zb_all[g][:, gsl].unsqueeze(2).broadcast_to((P, HI, BLOCK))
            yv = y[:].rearrange("p (h t) -> p h t", t=BLOCK)
            nc.vector.tensor_tensor(yv, s[:].rearrange(
                "p (h t) -> p h t", t=BLOCK), zb_b, op=ALU.mult)

            nc.sync.dma_start(
                out=y_d[b].rearrange("(p f) -> p f", p=P), in_=y[:])

    nc.compile()
    return nc


_NC_CACHE = {}


def _get_nc(NB, S, **kw):
    key = (NB, S, tuple(sorted(kw.items())))
    if key not in _NC_CACHE:
        _NC_CACHE[key] = build_core_kernel(NB, S, **kw)
    return _NC_CACHE[key]


def kernel(x: np.ndarray, control_params: np.ndarray):
    """Full-input entry: x (32,1,262144), control_params (32,2,262144).
    Returns (out, fc, q) matching reference."""
    from concourse.bass_utils import run_bass_kernel_spmd

    B, _, S = x.shape
    n_cores = 8
    nb = B // n_cores
    nblk = S // BLOCK
    nc = _get_nc(nb, S)
    consts = make_consts(nb, S)

    x2 = np.ascontiguousarray(x[:, 0, :], dtype=np.float32)
    cp = np.ascontiguousarray(control_params, dtype=np.float32)
    in_maps = [
        {"x": x2[c * nb:(c + 1) * nb], "cp": cp[c * nb:(c + 1) * nb], **consts}
        for c in range(n_cores)
    ]
    res = run_bass_kernel_spmd(nc, in_maps, list(range(n_cores)))

    out = np.empty((B, 1, S), dtype=np.float32)
    fc = np.empty((B, nblk), dtype=np.float32)
    q = np.empty((B, nblk), dtype=np.float32)
    for c in range(n_cores):
        rd = res.results[c]
        out[c * nb:(c + 1) * nb, 0, :] = rd["y"]
        fc[c * nb:(c + 1) * nb] = rd["fc"]
        q[c * nb:(c + 1) * nb] = rd["q"]
    return out, fc, q


# revision 19
# speedup vs baseline: 1.1245x; 1.1245x over previous
"""Trainium2 Bass kernel for nn_Lowpass: per-128-block RBJ lowpass biquad.

Algorithm (per 128-sample block, zero initial state):
  y = IIR(FIR(x)) with per-block coefficients from avg-pooled control params.
  FIR: u[n] = x[n] + 2 x[n-1] + x[n-2]    (b0 factored out; b2 == b0, b1 == 2 b0)
  IIR poles are complex (r e^{+-i theta}).  Rotated-frame decomposition turns
  the order-2 recurrence into two real first-order scans that map directly to
  the DVE tensor_tensor_scan instruction:
      v_re[n] = r v_re[n-1] + cos(n theta) u[n]
      v_im[n] = r v_im[n-1] - sin(n theta) u[n]
      y[n]    = Z b0 (cos(n theta + phi) v_re[n] - sin(n theta + phi) v_im[n])
  with 2c = 1 - i pr/pi the pole residue, Z = |2c|, phi = arg(2c).
  (The kernel scans d_im = +sin * u, flipping the recombine sign to +.)

Work distribution:
  PE:  angle grids n*theta/2pi (outer product theta'^T @ blockdiag(iota)),
       coefficient transposes.
  ACT: control-param avg-pooling (Abs + accum_out), magic-number rounding
       bias, all sin/cos/arctan evaluations (single trig table set).
  DVE: FIR, range-reduce subtract, the two scans, recombine multiplies,
       small coefficient arithmetic (sqrt via seeded Newton, no table switch).

Sharding: pure data parallel, core c processes batches [4c, 4c+4).
"""

import sys

sys.path.insert(0, "/opt/trn_rl_repo")

import math
from contextlib import ExitStack

import numpy as np

import concourse.bacc as bacc
import concourse.bass as bass
import concourse.mybir as mybir
from concourse.tile import TileContext

F32 = mybir.dt.float32
AX = mybir.AxisListType
ALU = mybir.AluOpType
ACT = mybir.ActivationFunctionType

SR = 44100.0
BLOCK = 128
FC_MIN, FC_MAX = 2000.0, 20000.0
Q_MIN, Q_MAX = 0.1, 10.0
PI = math.pi
MAGIC = 1.5 * 2.0 ** 23     # fp32 round-to-nearest-int bias
INV2PI = 1.0 / (2.0 * PI)
TWOPI = 2.0 * PI
# linear minimax seed for sqrt on [0.70, 0.87] (one Newton step after)
SQ_C1 = 0.5672
SQ_C0 = 0.4402


def make_consts(NB, S):
    """Host-precomputed constants: block-diag iota/ones rhs + identity."""
    P = 128
    F = S // P
    HI = F // BLOCK
    rhs = np.zeros((64, F), np.float32)
    for g in range(2):
        for h in range(HI):
            rhs[32 * g + h, h * BLOCK:(h + 1) * BLOCK] = np.arange(
                BLOCK, dtype=np.float32)
            rhs[32 * g + 16 + h, h * BLOCK:(h + 1) * BLOCK] = 1.0
    ident = np.eye(128, dtype=np.float32)
    return {"rhs_c": rhs, "ident": ident}


def build_core_kernel(NB=4, S=262144, n_devices=8, dev_clamp=False):
    """Bass kernel for one core: NB batches of S samples."""
    P = 128
    F = S // P            # free elems per row (per batch)
    HI = F // BLOCK       # blocks per partition row
    nblk = S // BLOCK     # blocks per batch
    NBG = (NB + 1) // 2   # coeff groups of 2 batches

    nc = bacc.Bacc("TRN2", target_bir_lowering=False, debug=False,
                   num_devices=n_devices)
    x_d = nc.dram_tensor("x", [NB, S], F32, kind="ExternalInput")
    cp_d = nc.dram_tensor("cp", [NB, 2, S], F32, kind="ExternalInput")
    rhs_d = nc.dram_tensor("rhs_c", [64, F], F32, kind="ExternalInput")
    id_d = nc.dram_tensor("ident", [128, 128], F32, kind="ExternalInput")
    y_d = nc.dram_tensor("y", [NB, S], F32, kind="ExternalOutput")
    fc_d = nc.dram_tensor("fc", [NB, nblk], F32, kind="ExternalOutput")
    q_d = nc.dram_tensor("q", [NB, nblk], F32, kind="ExternalOutput")

    with TileContext(nc) as tc, ExitStack() as ctx:
        cpool = ctx.enter_context(tc.tile_pool(name="const", bufs=1))
        spool = ctx.enter_context(tc.tile_pool(name="small", bufs=2))
        big = ctx.enter_context(tc.tile_pool(name="big", bufs=2))
        psum = ctx.enter_context(tc.tile_pool(name="psum", bufs=1,
                                              space="PSUM"))

        rhs_sb = cpool.tile([64, F], F32, tag="rhs_sb")
        nc.sync.dma_start(out=rhs_sb[:], in_=rhs_d[:, :])
        id_sb = cpool.tile([128, 128], F32, tag="id_sb")
        nc.sync.dma_start(out=id_sb[:], in_=id_d[:, :])

        _consts = {}

        def c_ap(val):
            if val not in _consts:
                t = cpool.tile([P, 1], F32, tag=f"c{len(_consts)}",
                               name=f"c{len(_consts)}")
                nc.vector.memset(t[:], val)
                _consts[val] = t
            return _consts[val][:]

        _bufs2 = {"x", "ang", "cpg", "spg", "cpt", "trash"}

        def bt(tag, shape=None):
            return big.tile(shape or [P, F], F32, tag=tag, name=tag,
                            bufs=2 if tag in _bufs2 else 1)

        # ---------- per-group coefficient pipeline ----------
        # group g covers batches 2g, 2g+1; W lanes per group per partition
        lhsT_g = []     # per group: (64,128) rows 32*(b%2)+[0:16]=theta',
        #                 +[16:32]=phi' (in turns)
        r_all, zb_all = [], []
        for g in range(NBG):
            bs = list(range(2 * g, min(2 * g + 2, NB)))
            W = HI * len(bs)

            def gt(tag):
                return spool.tile([P, W], F32, tag=tag, name=tag, bufs=2)

            s0, s1 = gt("s0"), gt("s1")
            for bi, b in enumerate(bs):
                for prm in range(2):
                    cpt = bt("cpt")
                    nc.sync.dma_start(
                        out=cpt[:],
                        in_=cp_d[b, prm].rearrange("(p f) -> p f", p=P))
                    dst = (s0 if prm == 0 else s1)
                    if g == 0:
                        # head: DVE is idle, use exact tensor_reduce
                        nc.vector.tensor_reduce(
                            dst[:, bi * HI:(bi + 1) * HI],
                            cpt[:].rearrange("p (h t) -> p h t", t=BLOCK),
                            axis=AX.X, op=ALU.add)
                    else:
                        # steady state: DVE busy streaming, pool on ACT
                        trash = bt("trash", [P, BLOCK])
                        for h in range(HI):
                            nc.scalar.activation(
                                trash[:], cpt[:, h * BLOCK:(h + 1) * BLOCK],
                                ACT.Abs,
                                accum_out=dst[:, bi * HI + h:bi * HI + h + 1])

            fc = gt("fc")
            nc.vector.tensor_scalar(fc[:], s0[:], (FC_MAX - FC_MIN) / BLOCK,
                                    FC_MIN, op0=ALU.mult, op1=ALU.add)
            q = gt("q")
            nc.vector.tensor_scalar(q[:], s1[:], (Q_MAX - Q_MIN) / BLOCK,
                                    Q_MIN, op0=ALU.mult, op1=ALU.add)
            for bi, b in enumerate(bs):
                nc.sync.dma_start(
                    out=fc_d[b].rearrange("(p h) -> p h", p=P),
                    in_=fc[:, bi * HI:(bi + 1) * HI])
                nc.sync.dma_start(
                    out=q_d[b].rearrange("(p h) -> p h", p=P),
                    in_=q[:, bi * HI:(bi + 1) * HI])

            w0 = gt("w0")
            nc.vector.tensor_scalar(
                w0[:], s0[:], (FC_MAX - FC_MIN) / BLOCK * 2.0 * PI / SR,
                FC_MIN * 2.0 * PI / SR, op0=ALU.mult, op1=ALU.add)
            sinw = gt("sinw")
            nc.scalar.activation(sinw[:], w0[:], ACT.Sin)
            cosw = gt("cosw")
            nc.scalar.activation(cosw[:], w0[:], ACT.Sin, scale=-1.0,
                                 bias=c_ap(PI / 2))

            qr = gt("qr")
            nc.vector.reciprocal(qr[:], q[:])
            alpha = gt("alpha")
            nc.vector.scalar_tensor_tensor(alpha[:], sinw[:], 0.5, qr[:],
                                           op0=ALU.mult, op1=ALU.mult)
            t0 = gt("t0")
            nc.vector.tensor_scalar_add(t0[:], alpha[:], 1.0)
            a0r = gt("a0r")
            nc.vector.reciprocal(a0r[:], t0[:])
            t1 = gt("t1")
            nc.vector.tensor_scalar(t1[:], cosw[:], -0.5, 0.5,
                                    op0=ALU.mult, op1=ALU.add)
            b0c = gt("b0c")
            nc.vector.tensor_tensor(b0c[:], t1[:], a0r[:], op=ALU.mult)
            prc = gt("prc")
            nc.vector.tensor_tensor(prc[:], cosw[:], a0r[:], op=ALU.mult)
            t2 = gt("t2")
            nc.vector.tensor_scalar(t2[:], alpha[:], -1.0, 1.0,
                                    op0=ALU.mult, op1=ALU.add)
            a2 = gt("a2")
            nc.vector.tensor_tensor(a2[:], t2[:], a0r[:], op=ALU.mult)
            prsq = gt("prsq")
            nc.vector.tensor_tensor(prsq[:], prc[:], prc[:], op=ALU.mult)
            pi2 = gt("pi2")
            nc.vector.tensor_tensor(pi2[:], a2[:], prsq[:], op=ALU.subtract)

            def sqrt_nr(dst_tag, a):
                # seeded Newton sqrt, valid on [0.70, 0.87]
                y0 = gt(dst_tag + "0")
                nc.vector.tensor_scalar(y0[:], a[:], SQ_C1, SQ_C0,
                                        op0=ALU.mult, op1=ALU.add)
                rc = gt(dst_tag + "r")
                nc.vector.reciprocal(rc[:], y0[:])
                th = gt(dst_tag + "h")
                nc.vector.scalar_tensor_tensor(th[:], a[:], 0.5, rc[:],
                                               op0=ALU.mult, op1=ALU.mult)
                out = gt(dst_tag)
                nc.vector.scalar_tensor_tensor(out[:], y0[:], 0.5, th[:],
                                               op0=ALU.mult, op1=ALU.add)
                return out

            r_t = sqrt_nr("rt", a2)
            pi_ = sqrt_nr("pit", pi2)
            pir = gt("pir")
            nc.vector.reciprocal(pir[:], pi_[:])
            ratio = gt("ratio")
            nc.vector.tensor_tensor(ratio[:], prc[:], pir[:], op=ALU.mult)
            atn = gt("atn")
            nc.scalar.activation(atn[:], ratio[:], ACT.Arctan, scale=-1.0)
            theta = gt("theta")
            nc.vector.tensor_scalar_add(theta[:], atn[:], PI / 2)
            cphi = gt("cphi")
            nc.scalar.activation(cphi[:], atn[:], ACT.Sin, bias=c_ap(PI / 2))
            z_t = gt("z_t")
            nc.vector.reciprocal(z_t[:], cphi[:])
            zb = gt("zb")
            nc.vector.tensor_tensor(zb[:], z_t[:], b0c[:], op=ALU.mult)
            r_all.append(r_t)
            zb_all.append(zb)

            # pack theta'/phi' (turns) for PE transpose; batch bi at cols
            # 32*bi + [0:HI) and 32*bi + 16 + [0:HI)
            tpin = spool.tile([128, 128], F32, tag="tpin", name="tpin",
                              bufs=2)
            nc.vector.memset(tpin[:], 0.0)
            for bi in range(len(bs)):
                nc.vector.tensor_scalar_mul(
                    tpin[:, 32 * bi:32 * bi + HI],
                    theta[:, bi * HI:(bi + 1) * HI], INV2PI)
                nc.vector.tensor_scalar_mul(
                    tpin[:, 32 * bi + 16:32 * bi + 16 + HI],
                    atn[:, bi * HI:(bi + 1) * HI], INV2PI)
            ps_t = psum.tile([128, 128], F32, tag="pa", name="ps_t")
            nc.tensor.transpose(ps_t[:], tpin[:], id_sb[:])
            lt = cpool.tile([64, 128], F32, tag=f"lhsT{g}", name=f"lhsT{g}")
            nc.scalar.copy(lt[:], ps_t[0:64, :])
            lhsT_g.append(lt)

        # ---------- per-batch streaming filter ----------
        for b in range(NB):
            g, bi = b // 2, b % 2
            gsl = slice(bi * HI, (bi + 1) * HI)
            lt = lhsT_g[g]
            p0 = 32 * bi

            x_sb = bt("x")
            nc.sync.dma_start(
                out=x_sb[:], in_=x_d[b].rearrange("(p f) -> p f", p=P))
            xv = x_sb[:].rearrange("p (h t) -> p h t", t=BLOCK)

            # FIR u = x + 2 x_{-1} + x_{-2} (per block; fix cols 0,1)
            u1 = bt("u1")
            nc.vector.scalar_tensor_tensor(
                u1[:, 1:], x_sb[:, :F - 1], 2.0, x_sb[:, 1:],
                op0=ALU.mult, op1=ALU.add)
            nc.vector.tensor_copy(u1[:, 0:1], x_sb[:, 0:1])
            u = bt("u")
            nc.vector.tensor_tensor(u[:, 2:], u1[:, 2:], x_sb[:, :F - 2],
                                    op=ALU.add)
            uv = u[:].rearrange("p (h t) -> p h t", t=BLOCK)
            nc.vector.tensor_copy(uv[:, :, 0:1], xv[:, :, 0:1])
            nc.vector.scalar_tensor_tensor(
                uv[:, :, 1:2], xv[:, :, 0:1], 2.0, xv[:, :, 1:2],
                op0=ALU.mult, op1=ALU.add)

            pa = psum.tile([P, F], F32, tag="pa", name="pa")
            pp = psum.tile([P, F], F32, tag="pp", name="pp")
            for c in range(0, F, 512):
                ce = min(c + 512, F)
                nc.tensor.matmul(pa[:, c:ce], lt[p0:p0 + 16, :],
                                 rhs_sb[p0:p0 + 16, c:ce])
                nc.tensor.matmul(pp[:, c:ce], lt[p0:p0 + 32, :],
                                 rhs_sb[p0:p0 + 32, c:ce])

            def grids(src, ctag, stag):
                # tk = MAGIC + round(src);  gn = (tk - MAGIC) - src = -frac
                tk = bt("tk")
                nc.scalar.activation(tk[:], src[:], ACT.Abs,
                                     bias=c_ap(MAGIC))
                gn = bt("g_" + stag)
                nc.vector.scalar_tensor_tensor(gn[:], tk[:], -MAGIC, src[:],
                                               op0=ALU.add, op1=ALU.subtract)
                if dev_clamp:
                    nc.vector.tensor_scalar(gn[:], gn[:], -0.5, 0.5,
                                            op0=ALU.max, op1=ALU.min)
                # sin(2 pi frac) = sin(-2 pi gn); cos = sin(pi/2 - 2 pi |gn|)
                sgr = bt(stag)
                nc.scalar.activation(sgr[:], gn[:], ACT.Sin, scale=-TWOPI)
                fa = bt("fa")
                nc.scalar.activation(fa[:], gn[:], ACT.Abs)
                cgr = bt(ctag)
                nc.scalar.activation(cgr[:], fa[:], ACT.Sin, scale=-TWOPI,
                                     bias=c_ap(PI / 2))
                return cgr, sgr

            cg, sg = grids(pa, "cg", "sg")      # cos/sin(n theta)
            cpg, spg = grids(pp, "cpg", "spg")  # cos/sin(n theta + phi)

            # scan multiplier grid: r per lane, 0 at block starts
            d0 = bt("d0")
            d0v = d0[:].rearrange("p (h t) -> p h t", t=BLOCK)
            r_b = r_all[g][:, gsl].unsqueeze(2).broadcast_to((P, HI, BLOCK))
            nc.scalar.activation(d0v, r_b, ACT.Copy)
            nc.vector.memset(d0v[:, :, 0:1], 0.0)

            dre = bt("dre")
            nc.vector.tensor_tensor(dre[:], cg[:], u[:], op=ALU.mult)
            dim = bt("dim")
            nc.vector.tensor_tensor(dim[:], sg[:], u[:], op=ALU.mult)

            vre = bt("vre")
            nc.vector.tensor_tensor_scan(vre[:], d0[:], dre[:], 0.0,
                                         op0=ALU.mult, op1=ALU.add)
            vim = bt("vim")
            nc.vector.tensor_tensor_scan(vim[:], d0[:], dim[:], 0.0,
                                         op0=ALU.mult, op1=ALU.add)

            # y = ZB * (cos(psi) v_re + sin(psi) v_im')   [v_im' = -v_im]
            m1 = bt("dre")
            nc.vector.tensor_tensor(m1[:], cpg[:], vre[:], op=ALU.mult)
            m2 = bt("dim")
            nc.vector.tensor_tensor(m2[:], spg[:], vim[:], op=ALU.mult)
            s = bt("u")
            nc.vector.tensor_tensor(s[:], m1[:], m2[:], op=ALU.add)
            y = bt("ang")
            zb_b = zb_all[g][:, gsl].unsqueeze(2).broadcast_to((P, HI, BLOCK))
            yv = y[:].rearrange("p (h t) -> p h t", t=BLOCK)
            nc.vector.tensor_tensor(yv, s[:].rearrange(
                "p (h t) -> p h t", t=BLOCK), zb_b, op=ALU.mult)

            nc.sync.dma_start(
                out=y_d[b].rearrange("(p f) -> p f", p=P), in_=y[:])

    nc.compile()
    return nc


_NC_CACHE = {}


def _get_nc(NB, S, **kw):
    key = (NB, S, tuple(sorted(kw.items())))
    if key not in _NC_CACHE:
        _NC_CACHE[key] = build_core_kernel(NB, S, **kw)
    return _NC_CACHE[key]


def kernel(x: np.ndarray, control_params: np.ndarray):
    """Full-input entry: x (32,1,262144), control_params (32,2,262144).
    Returns (out, fc, q) matching reference."""
    from concourse.bass_utils import run_bass_kernel_spmd

    B, _, S = x.shape
    n_cores = 8
    nb = B // n_cores
    nblk = S // BLOCK
    nc = _get_nc(nb, S)
    consts = make_consts(nb, S)

    x2 = np.ascontiguousarray(x[:, 0, :], dtype=np.float32)
    cp = np.ascontiguousarray(control_params, dtype=np.float32)
    in_maps = [
        {"x": x2[c * nb:(c + 1) * nb], "cp": cp[c * nb:(c + 1) * nb], **consts}
        for c in range(n_cores)
    ]
    res = run_bass_kernel_spmd(nc, in_maps, list(range(n_cores)))

    out = np.empty((B, 1, S), dtype=np.float32)
    fc = np.empty((B, nblk), dtype=np.float32)
    q = np.empty((B, nblk), dtype=np.float32)
    for c in range(n_cores):
        rd = res.results[c]
        out[c * nb:(c + 1) * nb, 0, :] = rd["y"]
        fc[c * nb:(c + 1) * nb] = rd["fc"]
        q[c * nb:(c + 1) * nb] = rd["q"]
    return out, fc, q


# revision 20
# speedup vs baseline: 1.1536x; 1.0259x over previous
"""Trainium2 Bass kernel for nn_Lowpass: per-128-block RBJ lowpass biquad.

Algorithm (per 128-sample block, zero initial state):
  y = IIR(FIR(x)) with per-block coefficients from avg-pooled control params.
  FIR: u[n] = x[n] + 2 x[n-1] + x[n-2]    (b0 factored out; b2 == b0, b1 == 2 b0)
  IIR poles are complex (r e^{+-i theta}).  Rotated-frame decomposition turns
  the order-2 recurrence into two real first-order scans that map directly to
  the DVE tensor_tensor_scan instruction:
      v_re[n] = r v_re[n-1] + cos(n theta) u[n]
      v_im[n] = r v_im[n-1] - sin(n theta) u[n]
      y[n]    = Z b0 (cos(n theta + phi) v_re[n] - sin(n theta + phi) v_im[n])
  with 2c = 1 - i pr/pi the pole residue, Z = |2c|, phi = arg(2c).
  (The kernel scans d_im = +sin * u, flipping the recombine sign to +.)

Work distribution:
  PE:  angle grids n*theta/2pi (outer product theta'^T @ blockdiag(iota)),
       coefficient transposes.
  ACT: control-param avg-pooling (Abs + accum_out), magic-number rounding
       bias, all sin/cos/arctan evaluations (single trig table set).
  DVE: FIR, range-reduce subtract, the two scans, recombine multiplies,
       small coefficient arithmetic (sqrt via seeded Newton, no table switch).

Sharding: pure data parallel, core c processes batches [4c, 4c+4).
"""

import sys

sys.path.insert(0, "/opt/trn_rl_repo")

import math
from contextlib import ExitStack

import numpy as np

import concourse.bacc as bacc
import concourse.bass as bass
import concourse.mybir as mybir
from concourse.tile import TileContext

F32 = mybir.dt.float32
AX = mybir.AxisListType
ALU = mybir.AluOpType
ACT = mybir.ActivationFunctionType

SR = 44100.0
BLOCK = 128
FC_MIN, FC_MAX = 2000.0, 20000.0
Q_MIN, Q_MAX = 0.1, 10.0
PI = math.pi
MAGIC = 1.5 * 2.0 ** 23     # fp32 round-to-nearest-int bias
INV2PI = 1.0 / (2.0 * PI)
TWOPI = 2.0 * PI
# linear minimax seed for sqrt on [0.70, 0.87] (one Newton step after)
SQ_C1 = 0.5672
SQ_C0 = 0.4402


def make_consts(NB, S):
    """Host-precomputed constants: block-diag iota/ones rhs + identity."""
    P = 128
    F = S // P
    HI = F // BLOCK
    rhs = np.zeros((64, F), np.float32)
    for g in range(2):
        for h in range(HI):
            rhs[32 * g + h, h * BLOCK:(h + 1) * BLOCK] = np.arange(
                BLOCK, dtype=np.float32)
            rhs[32 * g + 16 + h, h * BLOCK:(h + 1) * BLOCK] = 1.0
    ident = np.eye(128, dtype=np.float32)
    return {"rhs_c": rhs, "ident": ident}


def build_core_kernel(NB=4, S=262144, n_devices=8, dev_clamp=False):
    """Bass kernel for one core: NB batches of S samples."""
    P = 128
    F = S // P            # free elems per row (per batch)
    HI = F // BLOCK       # blocks per partition row
    nblk = S // BLOCK     # blocks per batch
    NBG = (NB + 1) // 2   # coeff groups of 2 batches

    nc = bacc.Bacc("TRN2", target_bir_lowering=False, debug=False,
                   num_devices=n_devices)
    x_d = nc.dram_tensor("x", [NB, S], F32, kind="ExternalInput")
    cp_d = nc.dram_tensor("cp", [NB, 2, S], F32, kind="ExternalInput")
    rhs_d = nc.dram_tensor("rhs_c", [64, F], F32, kind="ExternalInput")
    id_d = nc.dram_tensor("ident", [128, 128], F32, kind="ExternalInput")
    y_d = nc.dram_tensor("y", [NB, S], F32, kind="ExternalOutput")
    fc_d = nc.dram_tensor("fc", [NB, nblk], F32, kind="ExternalOutput")
    q_d = nc.dram_tensor("q", [NB, nblk], F32, kind="ExternalOutput")

    with TileContext(nc) as tc, ExitStack() as ctx:
        cpool = ctx.enter_context(tc.tile_pool(name="const", bufs=1))
        spool = ctx.enter_context(tc.tile_pool(name="small", bufs=2))
        big = ctx.enter_context(tc.tile_pool(name="big", bufs=2))
        psum = ctx.enter_context(tc.tile_pool(name="psum", bufs=1,
                                              space="PSUM"))

        rhs_sb = cpool.tile([64, F], F32, tag="rhs_sb")
        nc.sync.dma_start(out=rhs_sb[:], in_=rhs_d[:, :])
        id_sb = cpool.tile([128, 128], F32, tag="id_sb")
        nc.sync.dma_start(out=id_sb[:], in_=id_d[:, :])

        _consts = {}

        def c_ap(val):
            if val not in _consts:
                t = cpool.tile([P, 1], F32, tag=f"c{len(_consts)}",
                               name=f"c{len(_consts)}")
                nc.vector.memset(t[:], val)
                _consts[val] = t
            return _consts[val][:]

        _bufs2 = {"x", "ang", "cpg", "spg", "cpt", "trash"}

        def bt(tag, shape=None):
            return big.tile(shape or [P, F], F32, tag=tag, name=tag,
                            bufs=2 if tag in _bufs2 else 1)

        # ---------- per-group coefficient pipeline ----------
        # group g covers batches 2g, 2g+1; W lanes per group per partition
        lhsT_g = []     # per group: (64,128) rows 32*(b%2)+[0:16]=theta',
        #                 +[16:32]=phi' (in turns)
        r_all, zb_all = [], []
        _gsums = {}

        def gt_mk(W):
            def gt(tag):
                return spool.tile([P, W], F32, tag=tag, name=tag, bufs=2)
            return gt

        def pool_batch(g, bi, b, on_v):
            W = HI * 2
            if g not in _gsums:
                gt = gt_mk(W)
                _gsums[g] = (gt("s0"), gt("s1"))
            s0, s1 = _gsums[g]
            for prm in range(2):
                cpt = bt("cpt")
                nc.sync.dma_start(
                    out=cpt[:],
                    in_=cp_d[b, prm].rearrange("(p f) -> p f", p=P))
                dst = (s0 if prm == 0 else s1)
                if on_v:
                    nc.vector.tensor_reduce(
                        dst[:, bi * HI:(bi + 1) * HI],
                        cpt[:].rearrange("p (h t) -> p h t", t=BLOCK),
                        axis=AX.X, op=ALU.add)
                else:
                    trash = bt("trash", [P, BLOCK])
                    for h in range(HI):
                        nc.scalar.activation(
                            trash[:], cpt[:, h * BLOCK:(h + 1) * BLOCK],
                            ACT.Abs,
                            accum_out=dst[:, bi * HI + h:bi * HI + h + 1])

        def coeff_group(g):
            bs = list(range(2 * g, min(2 * g + 2, NB)))
            W = HI * len(bs)
            gt = gt_mk(W)
            s0, s1 = _gsums[g]

            fc = gt("fc")
            nc.vector.tensor_scalar(fc[:], s0[:], (FC_MAX - FC_MIN) / BLOCK,
                                    FC_MIN, op0=ALU.mult, op1=ALU.add)
            q = gt("q")
            nc.vector.tensor_scalar(q[:], s1[:], (Q_MAX - Q_MIN) / BLOCK,
                                    Q_MIN, op0=ALU.mult, op1=ALU.add)
            for bi, b in enumerate(bs):
                nc.sync.dma_start(
                    out=fc_d[b].rearrange("(p h) -> p h", p=P),
                    in_=fc[:, bi * HI:(bi + 1) * HI])
                nc.sync.dma_start(
                    out=q_d[b].rearrange("(p h) -> p h", p=P),
                    in_=q[:, bi * HI:(bi + 1) * HI])

            w0 = gt("w0")
            nc.vector.tensor_scalar(
                w0[:], s0[:], (FC_MAX - FC_MIN) / BLOCK * 2.0 * PI / SR,
                FC_MIN * 2.0 * PI / SR, op0=ALU.mult, op1=ALU.add)
            sinw = gt("sinw")
            nc.scalar.activation(sinw[:], w0[:], ACT.Sin)
            cosw = gt("cosw")
            nc.scalar.activation(cosw[:], w0[:], ACT.Sin, scale=-1.0,
                                 bias=c_ap(PI / 2))

            qr = gt("qr")
            nc.vector.reciprocal(qr[:], q[:])
            alpha = gt("alpha")
            nc.vector.scalar_tensor_tensor(alpha[:], sinw[:], 0.5, qr[:],
                                           op0=ALU.mult, op1=ALU.mult)
            t0 = gt("t0")
            nc.vector.tensor_scalar_add(t0[:], alpha[:], 1.0)
            a0r = gt("a0r")
            nc.vector.reciprocal(a0r[:], t0[:])
            t1 = gt("t1")
            nc.vector.tensor_scalar(t1[:], cosw[:], -0.5, 0.5,
                                    op0=ALU.mult, op1=ALU.add)
            b0c = gt("b0c")
            nc.vector.tensor_tensor(b0c[:], t1[:], a0r[:], op=ALU.mult)
            prc = gt("prc")
            nc.vector.tensor_tensor(prc[:], cosw[:], a0r[:], op=ALU.mult)
            t2 = gt("t2")
            nc.vector.tensor_scalar(t2[:], alpha[:], -1.0, 1.0,
                                    op0=ALU.mult, op1=ALU.add)
            a2 = gt("a2")
            nc.vector.tensor_tensor(a2[:], t2[:], a0r[:], op=ALU.mult)
            prsq = gt("prsq")
            nc.vector.tensor_tensor(prsq[:], prc[:], prc[:], op=ALU.mult)
            pi2 = gt("pi2")
            nc.vector.tensor_tensor(pi2[:], a2[:], prsq[:], op=ALU.subtract)

            def sqrt_nr(dst_tag, a):
                # seeded Newton sqrt, valid on [0.70, 0.87]
                y0 = gt(dst_tag + "0")
                nc.vector.tensor_scalar(y0[:], a[:], SQ_C1, SQ_C0,
                                        op0=ALU.mult, op1=ALU.add)
                rc = gt(dst_tag + "r")
                nc.vector.reciprocal(rc[:], y0[:])
                th = gt(dst_tag + "h")
                nc.vector.scalar_tensor_tensor(th[:], a[:], 0.5, rc[:],
                                               op0=ALU.mult, op1=ALU.mult)
                out = gt(dst_tag)
                nc.vector.scalar_tensor_tensor(out[:], y0[:], 0.5, th[:],
                                               op0=ALU.mult, op1=ALU.add)
                return out

            r_t = sqrt_nr("rt", a2)
            pi_ = sqrt_nr("pit", pi2)
            pir = gt("pir")
            nc.vector.reciprocal(pir[:], pi_[:])
            ratio = gt("ratio")
            nc.vector.tensor_tensor(ratio[:], prc[:], pir[:], op=ALU.mult)
            atn = gt("atn")
            nc.scalar.activation(atn[:], ratio[:], ACT.Arctan, scale=-1.0)
            theta = gt("theta")
            nc.vector.tensor_scalar_add(theta[:], atn[:], PI / 2)
            cphi = gt("cphi")
            nc.scalar.activation(cphi[:], atn[:], ACT.Sin, bias=c_ap(PI / 2))
            z_t = gt("z_t")
            nc.vector.reciprocal(z_t[:], cphi[:])
            zb = gt("zb")
            nc.vector.tensor_tensor(zb[:], z_t[:], b0c[:], op=ALU.mult)
            r_all.append(r_t)
            zb_all.append(zb)

            # pack theta'/phi' (turns) for PE transpose; batch bi at cols
            # 32*bi + [0:HI) and 32*bi + 16 + [0:HI)
            tpin = spool.tile([128, 128], F32, tag="tpin", name="tpin",
                              bufs=2)
            nc.vector.memset(tpin[:], 0.0)
            for bi in range(len(bs)):
                nc.vector.tensor_scalar_mul(
                    tpin[:, 32 * bi:32 * bi + HI],
                    theta[:, bi * HI:(bi + 1) * HI], INV2PI)
                nc.vector.tensor_scalar_mul(
                    tpin[:, 32 * bi + 16:32 * bi + 16 + HI],
                    atn[:, bi * HI:(bi + 1) * HI], INV2PI)
            ps_t = psum.tile([128, 128], F32, tag="pa", name="ps_t")
            nc.tensor.transpose(ps_t[:], tpin[:], id_sb[:])
            lt = cpool.tile([64, 128], F32, tag=f"lhsT{g}", name=f"lhsT{g}")
            nc.scalar.copy(lt[:], ps_t[0:64, :])
            lhsT_g.append(lt)

        # ---------- per-batch streaming filter ----------
        def stream_batch(b):
            g, bi = b // 2, b % 2
            gsl = slice(bi * HI, (bi + 1) * HI)
            lt = lhsT_g[g]
            p0 = 32 * bi

            x_sb = bt("x")
            nc.sync.dma_start(
                out=x_sb[:], in_=x_d[b].rearrange("(p f) -> p f", p=P))
            xv = x_sb[:].rearrange("p (h t) -> p h t", t=BLOCK)

            # FIR u = x + 2 x_{-1} + x_{-2} (per block; fix cols 0,1)
            u1 = bt("u1")
            nc.vector.scalar_tensor_tensor(
                u1[:, 1:], x_sb[:, :F - 1], 2.0, x_sb[:, 1:],
                op0=ALU.mult, op1=ALU.add)
            nc.vector.tensor_copy(u1[:, 0:1], x_sb[:, 0:1])
            u = bt("u")
            nc.vector.tensor_tensor(u[:, 2:], u1[:, 2:], x_sb[:, :F - 2],
                                    op=ALU.add)
            uv = u[:].rearrange("p (h t) -> p h t", t=BLOCK)
            nc.vector.tensor_copy(uv[:, :, 0:1], xv[:, :, 0:1])
            nc.vector.scalar_tensor_tensor(
                uv[:, :, 1:2], xv[:, :, 0:1], 2.0, xv[:, :, 1:2],
                op0=ALU.mult, op1=ALU.add)

            pa = psum.tile([P, F], F32, tag="pa", name="pa")
            pp = psum.tile([P, F], F32, tag="pp", name="pp")
            for c in range(0, F, 512):
                ce = min(c + 512, F)
                nc.tensor.matmul(pa[:, c:ce], lt[p0:p0 + 16, :],
                                 rhs_sb[p0:p0 + 16, c:ce])
                nc.tensor.matmul(pp[:, c:ce], lt[p0:p0 + 32, :],
                                 rhs_sb[p0:p0 + 32, c:ce])

            def grids(src, ctag, stag):
                # tk = MAGIC + round(src);  gn = (tk - MAGIC) - src = -frac
                tk = bt("tk")
                nc.scalar.activation(tk[:], src[:], ACT.Abs,
                                     bias=c_ap(MAGIC))
                gn = bt("g_" + stag)
                nc.vector.scalar_tensor_tensor(gn[:], tk[:], -MAGIC, src[:],
                                               op0=ALU.add, op1=ALU.subtract)
                if dev_clamp:
                    nc.vector.tensor_scalar(gn[:], gn[:], -0.5, 0.5,
                                            op0=ALU.max, op1=ALU.min)
                # sin(2 pi frac) = sin(-2 pi gn); cos = sin(pi/2 - 2 pi |gn|)
                sgr = bt(stag)
                nc.scalar.activation(sgr[:], gn[:], ACT.Sin, scale=-TWOPI)
                fa = bt("fa")
                nc.scalar.activation(fa[:], gn[:], ACT.Abs)
                cgr = bt(ctag)
                nc.scalar.activation(cgr[:], fa[:], ACT.Sin, scale=-TWOPI,
                                     bias=c_ap(PI / 2))
                return cgr, sgr

            cg, sg = grids(pa, "cg", "sg")      # cos/sin(n theta)
            cpg, spg = grids(pp, "cpg", "spg")  # cos/sin(n theta + phi)

            # scan multiplier grid: r per lane, 0 at block starts
            d0 = bt("d0")
            d0v = d0[:].rearrange("p (h t) -> p h t", t=BLOCK)
            r_b = r_all[g][:, gsl].unsqueeze(2).broadcast_to((P, HI, BLOCK))
            nc.scalar.activation(d0v, r_b, ACT.Copy)
            nc.vector.memset(d0v[:, :, 0:1], 0.0)

            dre = bt("dre")
            nc.vector.tensor_tensor(dre[:], cg[:], u[:], op=ALU.mult)
            dim = bt("dim")
            nc.vector.tensor_tensor(dim[:], sg[:], u[:], op=ALU.mult)

            vre = bt("vre")
            nc.vector.tensor_tensor_scan(vre[:], d0[:], dre[:], 0.0,
                                         op0=ALU.mult, op1=ALU.add)
            vim = bt("vim")
            nc.vector.tensor_tensor_scan(vim[:], d0[:], dim[:], 0.0,
                                         op0=ALU.mult, op1=ALU.add)

            # y = ZB * (cos(psi) v_re + sin(psi) v_im')   [v_im' = -v_im]
            m1 = bt("dre")
            nc.vector.tensor_tensor(m1[:], cpg[:], vre[:], op=ALU.mult)
            m2 = bt("dim")
            nc.vector.tensor_tensor(m2[:], spg[:], vim[:], op=ALU.mult)
            s = bt("u")
            nc.vector.tensor_tensor(s[:], m1[:], m2[:], op=ALU.add)
            y = bt("ang")
            zb_b = zb_all[g][:, gsl].unsqueeze(2).broadcast_to((P, HI, BLOCK))
            yv = y[:].rearrange("p (h t) -> p h t", t=BLOCK)
            nc.vector.tensor_tensor(yv, s[:].rearrange(
                "p (h t) -> p h t", t=BLOCK), zb_b, op=ALU.mult)

            nc.sync.dma_start(
                out=y_d[b].rearrange("(p f) -> p f", p=P), in_=y[:])

        # ---------- emission schedule (drives scheduler priority) ----------
        pool_batch(0, 0, 0, on_v=True)
        pool_batch(0, 1, 1, on_v=True)
        coeff_group(0)
        stream_batch(0)
        if NB > 2:
            pool_batch(1, 0, 2, on_v=False)
        stream_batch(1)
        if NB > 2:
            pool_batch(1, 1, 3, on_v=False)
            coeff_group(1)
            stream_batch(2)
            stream_batch(3)

    nc.compile()
    return nc


_NC_CACHE = {}


def _get_nc(NB, S, **kw):
    key = (NB, S, tuple(sorted(kw.items())))
    if key not in _NC_CACHE:
        _NC_CACHE[key] = build_core_kernel(NB, S, **kw)
    return _NC_CACHE[key]


def kernel(x: np.ndarray, control_params: np.ndarray):
    """Full-input entry: x (32,1,262144), control_params (32,2,262144).
    Returns (out, fc, q) matching reference."""
    from concourse.bass_utils import run_bass_kernel_spmd

    B, _, S = x.shape
    n_cores = 8
    nb = B // n_cores
    nblk = S // BLOCK
    nc = _get_nc(nb, S)
    consts = make_consts(nb, S)

    x2 = np.ascontiguousarray(x[:, 0, :], dtype=np.float32)
    cp = np.ascontiguousarray(control_params, dtype=np.float32)
    in_maps = [
        {"x": x2[c * nb:(c + 1) * nb], "cp": cp[c * nb:(c + 1) * nb], **consts}
        for c in range(n_cores)
    ]
    res = run_bass_kernel_spmd(nc, in_maps, list(range(n_cores)))

    out = np.empty((B, 1, S), dtype=np.float32)
    fc = np.empty((B, nblk), dtype=np.float32)
    q = np.empty((B, nblk), dtype=np.float32)
    for c in range(n_cores):
        rd = res.results[c]
        out[c * nb:(c + 1) * nb, 0, :] = rd["y"]
        fc[c * nb:(c + 1) * nb] = rd["fc"]
        q[c * nb:(c + 1) * nb] = rd["q"]
    return out, fc, q
